# revision 18
# baseline (speedup 1.0000x reference)
"""BNNLinear sampling kernel for Trainium2, data-parallel over 8 NeuronCores.

Computes h[m,c] = sum_r x_ext[m,r] * (mu[c,r] + sqrt(var[c,r]) * E[m,c,r])
with x_ext = concat([x, ones], axis=1), for
  x  [256, 512] f32, mu/var [512, 513] f32, E [256, 512, 513] f32.

Strategy (memory-bound; E is ~269 MB and must stream through HBM once):
 - Shard the sample axis m across the 8 cores (32 samples each).
 - Host-side LAYOUT ONLY: per-sample transpose of E to [r, c], blocked as
   [block, p, j, k, c] (r = 128k + p, sample m = 4*block + j) so each
   4-sample block is one contiguous 4 MB DMA (32 KB per partition);
   mu/var/x are pre-transposed the same way (tiny). All arithmetic
   (sqrt, multiplies, matmuls, sums) is on-chip.
 - Per sample: one DVE tensor_tensor B = E_t * sqrt(var)_t ([128, 2048],
   bf16 out), then 4 bf16 PE matmuls (stationary = x column chunk [128,1])
   accumulate sum_r into a PSUM row (4 samples per PSUM bank via
   tile_position col groups), plus a 5th matmul that adds the
   precomputed mu/bias row h1[m,:] via an identity-selector column.
 - h1 = x_t @ mu_t + mu bias row + sqrt(var_bias)*E_bias, computed once
   at setup ([32, C]); PE-broadcast of the sqrt(var) bias row.
 - PSUM banks are drained by ACT copies into an SBUF block laid out
   [32*(m%4) partition, m//4 block]; one final DMA scatters to DRAM.
"""

import numpy as np
from contextlib import ExitStack

import ml_dtypes

import concourse.bacc as bacc
import concourse.mybir as mybir
import concourse.tile as tile
from concourse.bass_utils import run_bass_kernel_spmd

F32 = mybir.dt.float32
BF16 = mybir.dt.bfloat16

N_CORES = 8
M_TOTAL = 256
M_SH = M_TOTAL // N_CORES  # 32 samples per core
C = 512
R_IN = 512                 # r chunks: 4 x 128
KCH = 4
GRP = 4                    # samples per DMA block
N_BLK = M_SH // GRP        # 8

_COMPILED = None


def _build_program(repeat=1, work_bufs=3, psum_bufs=4, bt_bufs=4, e7_bufs=2):
    nc = bacc.Bacc("TRN2", target_bir_lowering=False, debug=False)

    et_d = nc.dram_tensor("et", [N_BLK, 128, GRP, KCH, C], F32, kind="ExternalInput").ap()
    eb_d = nc.dram_tensor("eb", [M_SH, C], F32, kind="ExternalInput").ap()
    xt_d = nc.dram_tensor("xt", [128, KCH, M_SH], F32, kind="ExternalInput").ap()
    mu_d = nc.dram_tensor("mu_t", [128, KCH, C], F32, kind="ExternalInput").ap()
    mub_d = nc.dram_tensor("mu_b", [1, C], F32, kind="ExternalInput").ap()
    var_d = nc.dram_tensor("var_t", [128, KCH, C], F32, kind="ExternalInput").ap()
    varb_d = nc.dram_tensor("var_b", [1, C], F32, kind="ExternalInput").ap()
    id_d = nc.dram_tensor("ident", [M_SH, M_SH], BF16, kind="ExternalInput").ap()
    out_d = nc.dram_tensor("out", [M_SH, C], F32, kind="ExternalOutput").ap()

    with tile.TileContext(nc) as tc, ExitStack() as ctx:
        const = ctx.enter_context(tc.tile_pool(name="const", bufs=1))
        work = ctx.enter_context(tc.tile_pool(name="work", bufs=work_bufs))
        bpool = ctx.enter_context(tc.tile_pool(name="bpool", bufs=bt_bufs))
        opool = ctx.enter_context(tc.tile_pool(name="opool", bufs=1))
        e7pool = ctx.enter_context(tc.tile_pool(name="e7pool", bufs=e7_bufs))
        psum = ctx.enter_context(tc.tile_pool(name="psum", bufs=psum_bufs, space="PSUM"))
        psum1 = ctx.enter_context(tc.tile_pool(name="psum1", bufs=1, space="PSUM"))

        # ---- setup: constants in SBUF ----
        # Ordered so the DMA stream is: var (needed by every TT), x, first
        # E block, then the h1-path constants (needed only by the first
        # drain), then E blocks 1..7.
        s_sb = const.tile([128, KCH, C], F32)
        nc.sync.dma_start(s_sb[:], var_d)
        nc.scalar.sqrt(s_sb[:], s_sb[:])

        xt_sb = const.tile([128, KCH, M_SH], F32)
        nc.sync.dma_start(xt_sb[:], xt_d)
        xtb = const.tile([128, KCH, M_SH], BF16)
        nc.scalar.copy(xtb[:], xt_sb[:])
        varb_sb = const.tile([1, C], F32)
        nc.sync.dma_start(varb_sb[:], varb_d)
        sb_sb = const.tile([1, C], F32)
        nc.scalar.sqrt(sb_sb[:], varb_sb[:])
        sb_bf = const.tile([1, C], BF16)
        nc.scalar.copy(sb_bf[:], sb_sb[:])
        ident_sb = const.tile([M_SH, M_SH], BF16)
        nc.sync.dma_start(ident_sb[:], id_d)

        # first E block goes ahead of the h1-path constants
        eg0 = work.tile([128, GRP, KCH, C], F32, tag="eg")
        nc.sync.dma_start(eg0[:], et_d[0])

        mu_sb = const.tile([128, KCH, C], F32)
        nc.sync.dma_start(mu_sb[:], mu_d)
        mu_bf = const.tile([128, KCH, C], BF16)
        nc.scalar.copy(mu_bf[:], mu_sb[:])
        mub_sb = const.tile([1, C], F32)
        nc.sync.dma_start(mub_sb[:], mub_d)
        mub_bf = const.tile([1, C], BF16)
        nc.scalar.copy(mub_bf[:], mub_sb[:])
        eb_sb = const.tile([M_SH, C], F32)
        nc.sync.dma_start(eb_sb[:], eb_d)

        ones32 = const.tile([1, M_SH], BF16)
        nc.vector.memset(ones32[:], 1.0)

        # broadcast sqrt(var) bias row to 32 partitions via PE outer product
        ps_b = psum1.tile([M_SH, C], F32)
        nc.tensor.matmul(ps_b[:], lhsT=ones32[:], rhs=sb_bf[:], start=True, stop=True)
        sbb_sb = const.tile([M_SH, C], F32)
        nc.scalar.copy(sbb_sb[:], ps_b[:])

        # h1 = x_t @ mu_t + mu bias row  -> [32, 512] psum, rows = samples
        h1_ps = psum1.tile([M_SH, C], F32)
        for k in range(KCH):
            nc.tensor.matmul(
                h1_ps[:], lhsT=xtb[:, k, :], rhs=mu_bf[:, k, :],
                start=(k == 0), stop=False,
            )
        nc.tensor.matmul(h1_ps[:], lhsT=ones32[:], rhs=mub_bf[:], start=False, stop=True)

        # hbs[m, c] = h1[m, c] + Eb[m, c] * sqrt(var)[c, 512], cast to bf16
        # (added into each sample's PSUM row via an identity-selector matmul)
        ebs_sb = const.tile([M_SH, C], F32)
        nc.vector.tensor_tensor(
            out=ebs_sb[:], in0=eb_sb[:], in1=sbb_sb[:], op=mybir.AluOpType.mult
        )
        hbs_bf = const.tile([M_SH, C], BF16)
        nc.vector.tensor_tensor(
            out=hbs_bf[:], in0=h1_ps[:], in1=ebs_sb[:], op=mybir.AluOpType.add
        )

        out_sb = opool.tile([128, N_BLK, C], F32)

        # ---- main loop over blocks of 4 samples ----
        for r_i, b in [(rr, bb) for rr in range(repeat) for bb in range(N_BLK)]:
            # the last two blocks of each round run per-sample (and the
            # final sample per-chunk) so DVE trails the DMA stream by one
            # sample at round end, not one 4-sample block
            fine_blk = b >= N_BLK - 2
            last_blk = b == N_BLK - 1
            if r_i == 0 and b == 0:
                eg = eg0
            elif not fine_blk:
                eg = work.tile([128, GRP, KCH, C], F32, tag="eg")
                nc.sync.dma_start(eg[:], et_d[b])
            ps = psum.tile([128, C], F32, tag="ps")
            for g in range(GRP):
                m = GRP * b + g
                if last_blk and g == GRP - 1:
                    # last sample of the round streams per-chunk so the
                    # kernel tail is one chunk deep, not one sample
                    bt_chunks = []
                    for k in range(KCH):
                        e_c = bpool.tile([128, C], F32, tag="etc")
                        nc.sync.dma_start(e_c[:], et_d[b, :, g, k, :])
                        b_c = bpool.tile([128, C], BF16, tag="btc")
                        nc.vector.tensor_tensor(
                            out=b_c[:], in0=e_c[:], in1=s_sb[:, k, :],
                            op=mybir.AluOpType.mult,
                        )
                        bt_chunks.append(b_c)
                else:
                    if fine_blk:
                        # per-sample tiles: sample TTs start as each 1 MB
                        # lands instead of waiting for the whole block
                        e_in = e7pool.tile([128, KCH, C], F32, tag="e7")
                        nc.sync.dma_start(e_in[:], et_d[b, :, g, :, :])
                        e_in = e_in[:]
                    else:
                        e_in = eg[:, g, :, :]
                    bt = bpool.tile([128, KCH, C], BF16, tag="bt")
                    nc.vector.tensor_tensor(
                        out=bt[:], in0=e_in, in1=s_sb[:],
                        op=mybir.AluOpType.mult,
                    )
                    bt_chunks = [bt[:, k, :] for k in range(KCH)]
                for k in range(KCH):
                    nc.tensor.matmul(
                        ps[32 * g : 32 * g + 1, :],
                        lhsT=xtb[:, k, m : m + 1],
                        rhs=bt_chunks[k],
                        start=(k == 0),
                        stop=False,
                        tile_position=(0, 32 * g),
                    )
                # += hbs[m, :] (identity column m selects the row)
                nc.tensor.matmul(
                    ps[32 * g : 32 * g + 1, :],
                    lhsT=ident_sb[:, m : m + 1],
                    rhs=hbs_bf[:],
                    start=False,
                    stop=True,
                    tile_position=(0, 32 * g),
                )
            # drain bank b: rows {0,32,64,96} -> out_sb block b (ACT copy)
            nc.scalar.copy(out_sb[0:97, b, :], ps[0:97, :])

        nc.sync.dma_start(
            out_d.rearrange("(b g) c -> g b c", g=4),
            out_sb[0:128:32, :, :],
        )

    nc.compile()
    return nc


def _prep_inputs(x, mu, var, E):
    x = np.ascontiguousarray(x, dtype=np.float32)
    mu = np.ascontiguousarray(mu, dtype=np.float32)
    var = np.ascontiguousarray(var, dtype=np.float32)
    E = np.ascontiguousarray(E, dtype=np.float32)

    # mu/var transposed-blocked: [p, k, c] with r = 128k + p (r < 512)
    def blk(t):
        tt = np.ascontiguousarray(t.T[:R_IN])          # [512, 512] (r, c)
        return np.ascontiguousarray(
            tt.reshape(KCH, 128, C).transpose(1, 0, 2)  # [128, 4, 512]
        )

    mu_t = blk(mu)
    var_t = blk(var)
    mu_b = np.ascontiguousarray(mu[:, R_IN]).reshape(1, C)
    var_b = np.ascontiguousarray(var[:, R_IN]).reshape(1, C)
    ident = np.eye(M_SH, dtype=ml_dtypes.bfloat16)

    # E per-sample transpose + block: [m, p, k, c], r = 128k + p, then
    # grouped into 4-sample DMA blocks: [blk, p, j, k, c], m = 4*blk + j
    et = np.ascontiguousarray(
        E.transpose(0, 2, 1)[:, :R_IN, :]              # [256, 512(r), 512(c)]
        .reshape(M_TOTAL, KCH, 128, C)
        .transpose(0, 2, 1, 3)                          # [256, 128, 4, 512]
    )
    eb = np.ascontiguousarray(E[:, :, R_IN])            # [256, 512]

    # x transposed-blocked per core: [p, k, m_local]
    in_maps = []
    for core in range(N_CORES):
        sl = slice(core * M_SH, (core + 1) * M_SH)
        xs = x[sl]                                      # [32, 512]
        xt = np.ascontiguousarray(
            xs.T.reshape(KCH, 128, M_SH).transpose(1, 0, 2)  # [128, 4, 32]
        )
        et_g = np.ascontiguousarray(
            et[sl].reshape(N_BLK, GRP, 128, KCH, C).transpose(0, 2, 1, 3, 4)
        )                                               # [8, 128, 4, 4, 512]
        in_maps.append({
            "et": et_g,
            "eb": np.ascontiguousarray(eb[sl]),
            "xt": xt,
            "mu_t": mu_t,
            "var_t": var_t,
            "mu_b": mu_b,
            "var_b": var_b,
            "ident": ident,
        })
    return in_maps


def kernel(x, mu, var, E, shape=None, _trace=False, **_ignored):
    global _COMPILED
    if _COMPILED is None:
        _COMPILED = _build_program()
    nc = _COMPILED
    in_maps = _prep_inputs(np.asarray(x), np.asarray(mu), np.asarray(var), np.asarray(E))
    res = run_bass_kernel_spmd(
        nc, in_maps, core_ids=list(range(N_CORES)), trace=_trace,
    )
    out = np.concatenate([res.results[i]["out"] for i in range(N_CORES)], axis=0)
    if _trace:
        kernel._last_results = res
    return out
